# revision 17
# baseline (speedup 1.0000x reference)
"""CoxNAM Trainium2 kernel.

Computation (per feature f, for each batch row b):
    h1 = relu(x[b,f] * W1[f] + b1[f])        # [H1=256]
    h2 = relu(h1 @ W2[f] + b2[f])            # [H2=128]
    out[b] = sum_f (h2 @ W3[f] + b3[f])      # scalar

Sharding: features F=256 split across 8 NeuronCores (32 each, SPMD — one
program, per-core input shards). Per-core partials are summed on the host
along with sum(b3).

Per-core schedule (software-pipelined with lag 1 over feature groups g):
  q-rounds over 1024-wide batch slices; within a round, window g issues
    A (PE):  L1 for group g: z1[h,b] = W1*x + b1 as K=2 matmuls, 4 features
             packed in the 4 PE row-groups, N=512 -> za [128,2048] (4 banks)
    B (DVE+ACT): relu(za) -> h1 bf16, each za tile split in half so both
             engines drain one tile concurrently
    C (PE):  L2 for group g-1: z2 = W2^T h1 (K=256 in 2 accumulated chunks)
             into zc [128,1024] (both 512-wide batch halves of the round)
    D (DVE/ACT): t = relu(zc + b2) -> tt bf16, alternating engines
    E (PE):  L3: acc[32j] += W3^T t, M=1 matmuls col-packed 4-wide,
             accumulated in PSUM over all g; full-height copy + strided DMA
             drains the 4 rows per round.
  DVE+ACT are the roofline (~1 elem/cycle/lane PSUM reads); PE work is
  interleaved so it never gates the drains and stays at high p-state.
"""

import os

import numpy as np
import ml_dtypes

F, B, H1, H2 = 256, 4096, 256, 128
NCORES = 8
BT = 512  # batch-tile width (one PSUM bank of fp32)
HC = H1 // 128  # h-chunks per feature
JW = 4  # feature packing width (PE row/col groups)
QB = 2 * BT  # batch per q-round (za slot width / zc width)

_CACHE = {}


def _jax_cache_setup():
    import jax

    d = os.path.join(os.path.expanduser("~"), ".cache", "coxnam_jaxcache")
    os.makedirs(d, exist_ok=True)
    jax.config.update("jax_compilation_cache_dir", d)
    jax.config.update("jax_persistent_cache_min_compile_time_secs", 0.0)
    jax.config.update("jax_persistent_cache_min_entry_size_bytes", 0)


def build_nc(fl=F // NCORES, b=B, dtype_name="bf16"):
    """Build the SPMD Bass program for one core holding `fl` features."""
    from contextlib import ExitStack

    import concourse.mybir as mybir
    import concourse.tile as tile
    from concourse import bacc

    dt = mybir.dt
    sdt = dt.bfloat16 if dtype_name == "bf16" else dt.float32
    f32r = dtype_name == "f32r"
    nq = b // QB  # q-rounds (4)
    ng = fl // JW  # feature groups of 4 (8)
    assert fl % JW == 0 and b % QB == 0

    nc = bacc.Bacc("TRN2", target_bir_lowering=False, debug=False)
    # compact per-feature row pairs: row 2f = x_f / W1_f, row 2f+1 = ones/b1_f
    xc = nc.dram_tensor("xc", [2 * fl, b], sdt, kind="ExternalInput").ap()
    w1c = nc.dram_tensor("w1c", [2 * fl, H1], sdt, kind="ExternalInput").ap()
    w2r = nc.dram_tensor("w2r", [128, fl * HC * H2], sdt, kind="ExternalInput").ap()
    b2t = nc.dram_tensor("b2t", [H2, fl], dt.float32, kind="ExternalInput").ap()
    w3 = nc.dram_tensor("w3", [H2, fl], sdt, kind="ExternalInput").ap()
    out = nc.dram_tensor("out", [JW, b], dt.float32, kind="ExternalOutput").ap()

    Relu = mybir.ActivationFunctionType.Relu
    add_, max_ = mybir.AluOpType.add, mybir.AluOpType.max

    def mm(ap):
        return ap.bitcast(dt.float32r) if f32r else ap

    # greedy DVE/ACT balancing for the PSUM-read epilogues
    ns = {"v": 0.0, "s": 0.0}

    def balanced(kind, out_ap, in_ap, bias_ap, width):
        # per-op costs calibrated from NTFF traces of this kernel
        tv = (175 + width) / 0.96
        ts = (312 + width) / 1.2
        use_v = ns["v"] + tv <= ns["s"] + ts
        if use_v:
            ns["v"] += tv
        else:
            ns["s"] += ts
        if kind == "relu":
            if use_v:
                nc.vector.tensor_scalar_max(out_ap, in_ap, 0.0)
            else:
                nc.scalar.activation(out_ap, in_ap, Relu)
        elif kind == "bias_relu":
            if use_v:
                nc.vector.tensor_scalar(out_ap, in_ap, bias_ap, 0.0, op0=add_, op1=max_)
            else:
                nc.scalar.activation(out_ap, in_ap, Relu, bias=bias_ap)
        else:  # copy
            if use_v:
                nc.vector.tensor_copy(out_ap, in_ap)
            else:
                nc.scalar.copy(out_ap, in_ap)

    with tile.TileContext(nc) as tc, ExitStack() as ctx:
        const = ctx.enter_context(tc.tile_pool(name="const", bufs=1))
        # xg[g]: feature 4g+j's x row at partition 32j, ones at 32j+1
        xg = [const.tile([128, b], sdt, name=f"xg{g}") for g in range(ng)]
        w1g = [const.tile([128, H1], sdt, name=f"w1g{g}") for g in range(ng)]
        w2s = const.tile([128, fl * HC * H2], sdt, name="w2s")
        b2s = const.tile([H2, fl], dt.float32, name="b2s")
        w3s = const.tile([H2, fl], sdt, name="w3s")

        w2chunk = JW * HC * H2
        # only the 2 used rows of each 32-row group are DMA'd (partition-
        # strided: x rows land at 32j, ones rows at 32j+1); g0 first so the
        # first A matmuls start immediately; f-split g0's W2 chunk likewise
        for g in range(ng):
            for r in range(2):
                nc.sync.dma_start(
                    w1g[g][r : 97 + r : 32, :], w1c[8 * g + r : 8 * g + 8 : 2, :]
                )
                nc.sync.dma_start(
                    xg[g][r : 97 + r : 32, :], xc[8 * g + r : 8 * g + 8 : 2, :]
                )
            if g == 0:
                nc.sync.dma_start(b2s[:], b2t[:])
                nc.sync.dma_start(w3s[:], w3[:])
            sub = w2chunk // JW
            for ff in range(JW if g == 0 else 1):
                w = sub if g == 0 else w2chunk
                nc.sync.dma_start(
                    w2s[:, g * w2chunk + ff * sub : g * w2chunk + ff * sub + w],
                    w2r[:, g * w2chunk + ff * sub : g * w2chunk + ff * sub + w],
                )

        # PSUM: za 2x2 banks (ping-pong) + zc 2x1 banks + pes 2 banks = 8
        pa = ctx.enter_context(tc.tile_pool(name="pa", bufs=2, space="PSUM"))
        pc = ctx.enter_context(tc.tile_pool(name="pc", bufs=2, space="PSUM"))
        pe = ctx.enter_context(tc.tile_pool(name="pe", bufs=1, space="PSUM"))
        hp = ctx.enter_context(tc.tile_pool(name="hp", bufs=17, space="SBUF"))
        tp = ctx.enter_context(tc.tile_pool(name="tp", bufs=10, space="SBUF"))
        op = ctx.enter_context(tc.tile_pool(name="op", bufs=2, space="SBUF"))

        def l1_slot(g, q, hc, bt):
            """One j-pair of L1 matmuls -> za [128, 1024] ping-pong tile."""
            bs = slice(q * QB + bt * BT, q * QB + (bt + 1) * BT)
            hts = []
            for p in range(2):
                za = pa.tile([128, 2 * BT], dt.float32, tag="za", name="za")
                for i in range(2):
                    j = 2 * p + i
                    nc.tensor.matmul(
                        za[:, i * BT : (i + 1) * BT],
                        mm(w1g[g][32 * j : 32 * j + 2, hc * 128 : hc * 128 + 128]),
                        mm(xg[g][32 * j : 32 * j + 2, bs]),
                        start=True,
                        stop=True,
                        tile_position=(32 * j, 0),
                    )
                ht = hp.tile([128, 2 * BT], sdt, tag="ht", name="ht")
                balanced("relu", ht[:], za[:], None, 2 * BT)
                hts.append(ht)
            return hts

        def l2_f(g, q, j, hts):
            """L2 matmuls + bias_relu epilogue for feature 4g+j."""
            f = JW * g + j
            p, i = divmod(j, 2)
            tts = []
            for bt in range(2):
                zc = pc.tile([H2, BT], dt.float32, tag="zc", name="zc")
                for hc in range(HC):
                    nc.tensor.matmul(
                        zc[:],
                        mm(w2s[:, (f * HC + hc) * H2 : (f * HC + hc + 1) * H2]),
                        mm(hts[hc, bt][p][:, i * BT : (i + 1) * BT]),
                        start=(hc == 0),
                        stop=(hc == HC - 1),
                    )
                tt = tp.tile([H2, BT], sdt, tag="tt", name="tt")
                balanced("bias_relu", tt[:], zc[:], b2s[:, f : f + 1], BT)
                tts.append(tt)
            return tts

        def l3_f(g, j, tts, pes):
            for bt in range(2):
                nc.tensor.matmul(
                    pes[32 * j : 32 * j + 1, bt * BT : (bt + 1) * BT],
                    mm(w3s[:, JW * g + j : JW * g + j + 1]),
                    mm(tts[bt][:]),
                    start=(g == 0),
                    stop=(g == ng - 1),
                    tile_position=(0, 32 * j),
                )

        # flat software pipeline over (q, g) windows: window w produces L1
        # for (q, g), consumes L2 for the previous window and L3 (packed
        # 4-wide, no tt wait) for the window before that
        pes_by_q = {}
        prev = None
        l3q = []  # pending (q, g, tts-per-feature)
        for w in range(nq * ng + 2):
            cur = divmod(w, ng) if w < nq * ng else None
            hts = {}
            tts = []
            for i in range(4):
                hc, bt = divmod(i, 2)
                if cur is not None:
                    hts[hc, bt] = l1_slot(cur[1], cur[0], hc, bt)
                if i == 0:
                    if l3q:
                        lq, lg, ltts = l3q.pop(0)
                        for j in range(4):
                            l3_f(lg, j, ltts[j], pes_by_q[lq])
                        if lg == ng - 1:
                            ot = op.tile([128, QB], dt.float32, tag="ot", name="ot")
                            balanced("copy", ot[:], pes_by_q[lq][:], None, QB)
                            nc.sync.dma_start(
                                out[:, lq * QB : (lq + 1) * QB], ot[0:128:32, :]
                            )
                    if prev is not None and prev[1] == 0:
                        # created after the prior quarter's drain was issued
                        t = pe.tile([128, QB], dt.float32, tag="pes", name=f"p{prev[0]}")
                        # full-height drain reads rows the E-matmuls never write
                        nc.vector.memset(t[:], 0.0)
                        pes_by_q[prev[0]] = t
                if prev is not None:
                    tts.append(l2_f(prev[1], prev[0], i, prev[2]))
            if prev is not None:
                l3q.append((prev[0], prev[1], tts))
            prev = (cur[0], cur[1], hts) if cur is not None else None

    nc.compile()
    return nc


def make_in_maps(x, W1, b1, W2, b2, W3, ncores=NCORES, dtype_name="bf16"):
    """Host-side shard + layout prep. Inputs are np.float32 full tensors."""
    fl = F // ncores
    npdt = ml_dtypes.bfloat16 if dtype_name == "bf16" else np.float32
    W1f = W1.reshape(F, H1)
    W3f = W3.reshape(F, H2)

    def cast(a):
        return np.ascontiguousarray(a).astype(npdt)

    in_maps = []
    for c in range(ncores):
        fs = slice(c * fl, (c + 1) * fl)
        xcm = np.empty((2 * fl, x.shape[0]), dtype=npdt)
        xcm[0::2] = cast(x[:, fs].T)
        xcm[1::2] = npdt(1.0)
        w1cm = np.empty((2 * fl, H1), dtype=npdt)
        w1cm[0::2] = cast(W1f[fs])
        w1cm[1::2] = cast(b1[fs])
        # w2r[p, (f*HC+hc)*H2+k] = W2[f, hc*128+p, k]
        w2r_c = (
            W2[fs]
            .reshape(fl, HC, 128, H2)
            .transpose(2, 0, 1, 3)
            .reshape(128, fl * HC * H2)
        )
        in_maps.append(
            {
                "xc": xcm,
                "w1c": w1cm,
                "w2r": cast(w2r_c),
                "b2t": np.ascontiguousarray(b2[fs].T, dtype=np.float32),
                "w3": cast(W3f[fs].T),
            }
        )
    return in_maps


def kernel(x, W1, b1, W2, b2, W3, b3, _trace=False):
    _jax_cache_setup()
    from concourse.bass_utils import run_bass_kernel_spmd

    x = np.asarray(x, dtype=np.float32)
    W1 = np.asarray(W1, dtype=np.float32)
    b1 = np.asarray(b1, dtype=np.float32)
    W2 = np.asarray(W2, dtype=np.float32)
    b2 = np.asarray(b2, dtype=np.float32)
    W3 = np.asarray(W3, dtype=np.float32)
    b3 = np.asarray(b3, dtype=np.float32)

    if "nc" not in _CACHE:
        _CACHE["nc"] = build_nc()
    nc = _CACHE["nc"]

    in_maps = make_in_maps(x, W1, b1, W2, b2, W3)
    res = run_bass_kernel_spmd(nc, in_maps, core_ids=list(range(NCORES)), trace=_trace)
    total = np.zeros(B, dtype=np.float64)
    for c in range(NCORES):
        total += res.results[c]["out"].astype(np.float64).sum(axis=0)
    total += float(b3.sum())
    outv = total.astype(np.float32)[:, None]
    if _trace:
        kernel.last_results = res
    return outv
